# revision 23
# baseline (speedup 1.0000x reference)
"""AttentiveStatisticsPooling Trainium2 Bass kernel (v3).

Self-contained: builds + compiles + runs an 8-core SPMD Bass program.

Math (faithful to the reference module, including its x - mean**2 quirk):
  T_n     = #{l : l < lengths[n]*L}                     (exact fp32 compare)
  mean_g  = sum_{l<T} x / T                             [N, C]
  std_g   = sqrt(clamp(mean_g - mean_g^2, EPS))         (weights sum to 1)
  hfin    = max(tanh(s*h + (s*c + t)), tanh(t))         == tanh(s*relu(h+c)+t)
            since s>0 and tanh monotone; h = W1a@x, c = W1b@mg+W1c@sg+b1
  a       = W2@hfin  (b2 dropped: softmax-invariant; zero anyway)
  e       = exp(a)   NO mask: tail columns (x pre-zeroed) all produce the
            same e0[c] = exp((W2@hfin0)[c]); subtract cnt*e0 from sum(e).
  mean    = sum_{l<T} e*x / (sum_l e - cnt*e0)          [N, C]
  std     = sqrt(clamp(mean - mean^2, EPS))
  out     = concat(mean, std)[:, :, None]               [N, 2C, 1]

Sharding: data-parallel over N; 16 samples -> 8 cores x 2 slots (sorted by T,
slot0 = 8 longest). Static slot widths = max T per slot, padded to mult of 4.

v3 engine split (profile-driven; DVE was the v2 bottleneck at 93% busy —
the reduction/accum DVE ops run at 1x, NOT the assumed 2x/4x modes):
  PE:   h+a matmuls (f16, 512-col blocks)                       ~32us
  Act:  per-block tanh; per-chunk-pair exact exp with hardware
        accum -> sum(e) (+290ns accumulator read each); slot1
        sum-x as Identity+accum; bias add fused into Identity   ~36us
  DVE:  fused e*x product+reduce via SCALAR_TENSOR_TENSOR (1x,
        ~1.3us/1024-chunk; replaces v2's TT + cache-reduce at
        1.9us); hfin max (4x); stats/final chains               ~38us
  Pool: slot0 sum-x fold tree (2 halving TTs W->W/4) + short DVE
        cache-reduce (Pool>19us busy inflates DVE ops ~25% via
        the shared SBUF port — why slot1 rides Act instead)     ~19us
  (tensor_tensor_reduce would fuse e*x+reduce at one op but crashes the
  runtime — probed on HW; SWDGE accum-DMA folds race per-dest: also dead.)

Loop: 3-stage For_i_pipelined (prep: loads+sum-x+stats | blocks | finals),
unroll=4, staggered barrier reset. Within the block stage the 5 chunk-pairs
are emitted SOFTWARE-PIPELINED with LEAD=2: pair k+2's h-matmul/tanh/hfin
enter the engine queues before pair k's a-matmul/exp/e*x-reduce, killing
the head-of-line stalls of the in-order engine queues (59.7 -> 39.9us/iter;
LEAD=3 regresses: h_ps PSUM ring (2 bufs) serializes the heads).
DMA (x stream ~5MB/iter) is ~25-45% occupied — NOT the bottleneck (v2's
docstring was wrong about that; profiles show DVE-bound + queue stalls).
"""

import numpy as np
import ml_dtypes

N, C, L, A = 16, 512, 3000, 128
NCORES = 8
CC = C // 128          # 4 channel chunks of 128 partitions
BLK = 512              # l-block width (one fp32 PSUM bank)
EPS = 1e-12
RSQRT_MAGIC = float(0x5F3759DF)

F16 = np.float16

# Schraudolph exp in fp16-bit domain: bits = a*SCH_A + SCH_B, bitcast to f16.
SCH_A = 1024.0 / float(np.log(2.0))        # 2^10 / ln 2
SCH_B = 15360.0 - 59.4                     # (15<<10) - RMS-optimal correction

OPTS = {
    "dve_exp": 0,         # non-tail chunk-pairs on DVE (Schraudolph)
    "se_dve": 0,          # act-exps whose sum-e rides a DVE pass instead
    "exact_exp_only": False,   # force all exp on Act (accuracy fallback)
    "use_stt": 1,         # fused e*x product+reduce (SCALAR_TENSOR_TENSOR)
    "sumx_mode": "mix",   # "cr" DVE | "pool" GpSimd fold | "mix" pool+Act
    "x_dtype": "f16",     # "f16" | "bf16"
    "stats_newton": 0,    # NR iters for std_g (bias path only; seed ~3%)
    "final_newton": 1,    # NR iters for the output std
    "pipe_unroll": 4,     # For_i_pipelined ticks per all-engine barrier
    "probe": 0,           # 1 = timing probe (skip block phase; WRONG output)
    "loop_hints": 1,      # branch-prefetch hints on the loop back-edge
    "stagger": 1,         # staggered semaphore reset at the loop barrier
    "dma_rings": 1,       # 1 = all x loads on SP; 2 = split SP/Act rings
    "xmerge": 0,          # one merged DMA per slot instead of 4 chunk DMAs
    "x1_bufs4": 0,        # 4 buffers for slot1 x (3-tick DMA lead)
}


# ---------------------------------------------------------------- host prep

def _lengths_to_T(lengths):
    """Exact replica of the reference fp32 mask comparison."""
    idx = np.arange(L, dtype=np.float32)
    thresh = (lengths.astype(np.float32) * np.float32(L)).astype(np.float32)
    return (idx[None, :] < thresh[:, None]).sum(axis=1).astype(np.int64)


def _host_prep(x, lengths, W1, b1, bn_gamma, bn_beta, bn_mean, bn_var, W2, b2):
    x = np.asarray(x)
    xdt = F16 if OPTS["x_dtype"] == "f16" else ml_dtypes.bfloat16
    Ts = np.maximum(_lengths_to_T(np.asarray(lengths)), 1)
    order = np.argsort(-Ts, kind="stable")
    slots = [order[:NCORES], order[NCORES:]]
    # pad widths to a multiple of 4 so the Pool sum-x fold tree divides evenly
    widths = [-4 * (-int(Ts[s].max()) // 4) for s in slots]
    tmins = [int(Ts[s].min()) for s in slots]

    def chunk_cols(m):  # [C, A] -> [128, CC*A], chunk cc at cols [cc*A:(cc+1)*A]
        return np.ascontiguousarray(
            m.reshape(CC, 128, m.shape[1]).transpose(1, 0, 2).reshape(128, -1))

    s = (np.asarray(bn_gamma) / np.sqrt(np.asarray(bn_var) + 1e-5)).astype(np.float32)
    t = (np.asarray(bn_beta) - np.asarray(bn_mean) * s).astype(np.float32)
    assert (s > 0).all(), "tanh/max fusion requires positive BN scale"
    W1 = np.asarray(W1, dtype=np.float32)
    W2 = np.asarray(W2, dtype=np.float32)
    # fold s into the context-conv weights so cvec = s*c + t needs no extra op
    w1b_s = W1[:, C:2 * C] * s[:, None]
    w1c_s = W1[:, 2 * C:] * s[:, None]
    sb1 = (s * np.asarray(b1, dtype=np.float32) + t).astype(np.float32)

    shared = {
        "w1aT": chunk_cols(np.ascontiguousarray(W1[:, :C].T)).astype(xdt),
        "w2T":  np.ascontiguousarray(W2.T).astype(xdt),           # [A, C]
        "w1bT": chunk_cols(np.ascontiguousarray(w1b_s.T)).astype(np.float32),
        "w1cT": chunk_cols(np.ascontiguousarray(w1c_s.T)).astype(np.float32),
        "svec": s.reshape(A, 1),
        "sb1v": sb1.reshape(A, 1),
        "ttv":  np.tanh(t).reshape(A, 1).astype(np.float32),
    }

    in_maps, metas = [], []
    for core in range(NCORES):
        m = dict(shared)
        meta = []
        misc = np.zeros((128, 4), dtype=np.float32)
        invT8 = np.zeros((128, 8), dtype=np.float32)
        cnt8 = np.zeros((128, 8), dtype=np.float32)
        for sl in range(2):
            n = int(slots[sl][core])
            T = int(Ts[n])
            W = widths[sl]
            xb = x[n, :, :W].astype(xdt)
            if T < W:
                xb[:, T:] = xdt(0)
            m[f"x{sl}"] = xb
            misc[:, sl] = 1.0 / T
            misc[:, 2 + sl] = float(W - T)
            invT8[:, sl * 4:sl * 4 + 4] = 1.0 / T
            cnt8[:, sl * 4:sl * 4 + 4] = float(W - T)
            meta.append((n, T))
        m["misc"] = misc
        m["invT8"] = invT8
        m["cnt8"] = cnt8
        in_maps.append(m)
        metas.append(meta)
    return in_maps, metas, widths, tmins


# ---------------------------------------------------------------- program

def _build_program(widths, tmins, loop=False, unroll=1):
    import concourse.bass as bass  # noqa: F401
    import concourse.tile as tile
    from concourse import bacc, mybir
    from contextlib import ExitStack

    f32, i32, i16 = mybir.dt.float32, mybir.dt.int32, mybir.dt.int16
    xdt = mybir.dt.float16 if OPTS["x_dtype"] == "f16" else mybir.dt.bfloat16
    Alu = mybir.AluOpType
    Act = mybir.ActivationFunctionType

    nblk = [(w + BLK - 1) // BLK for w in widths]
    npair = [(nb + 1) // 2 for nb in nblk]

    # ---- static routing tables --------------------------------------
    # exp route per (sl, b, cc): blocks overlapping [tmin, W) must use the
    # exact Act exp (tail columns contribute cnt*exp(z) to the correction).
    # Emission interleaves slots pair-by-pair; build the same order here so
    # pool work spreads evenly over the pipeline.
    emit_order = []
    for p in range(max(npair)):
        for sl in range(2):
            if p < npair[sl]:
                emit_order.append((sl, p))
    # exp route per (sl, p, cc) chunk-pair: pairs overlapping [tmin, W) must
    # use exact Act exp; free pairs may use the DVE Schraudolph.
    exp_route = {}
    free_list = []
    for sl, p in emit_order:
        tail = min(2 * (p + 1) * BLK, widths[sl]) > tmins[sl]
        for cc in range(CC):
            if tail or OPTS["exact_exp_only"]:
                exp_route[(sl, p, cc)] = "act"
            else:
                free_list.append((sl, p, cc))
    dve_n = 0 if OPTS["exact_exp_only"] else OPTS["dve_exp"]
    nfree = max(len(free_list), 1)
    for i, key in enumerate(free_list):
        exp_route[key] = "dve" if (i * dve_n) // nfree != ((i + 1) * dve_n) // nfree else "act"
    # Σe of act-routed exps: a few ride a DVE pass instead of the Act
    # accumulator (saves the 187ns read-accumulator slot on Act)
    act_keys = [k for sl, p in emit_order for k in
                [(sl, p, cc) for cc in range(CC)] if exp_route[k] == "act"]
    nact = max(len(act_keys), 1)
    sedve_n = OPTS["se_dve"]
    sedve = {k for i, k in enumerate(act_keys)
             if (i * sedve_n) // nact != ((i + 1) * sedve_n) // nact}

    nc = bacc.Bacc("TRN2", target_bir_lowering=False, debug=False,
                   num_devices=NCORES)
    reps = (nc.dram_tensor("reps", [1, 1], i32, kind="ExternalInput").ap()
            if loop else None)

    xs = [nc.dram_tensor(f"x{sl}", [C, widths[sl]], xdt,
                         kind="ExternalInput").ap() for sl in range(2)]
    misc = nc.dram_tensor("misc", [128, 4], f32, kind="ExternalInput").ap()
    invT8 = nc.dram_tensor("invT8", [128, 8], f32, kind="ExternalInput").ap()
    cnt8 = nc.dram_tensor("cnt8", [128, 8], f32, kind="ExternalInput").ap()
    w1aT = nc.dram_tensor("w1aT", [128, CC * A], xdt, kind="ExternalInput").ap()
    w2T = nc.dram_tensor("w2T", [A, C], xdt, kind="ExternalInput").ap()
    w1bT = nc.dram_tensor("w1bT", [128, CC * A], f32, kind="ExternalInput").ap()
    w1cT = nc.dram_tensor("w1cT", [128, CC * A], f32, kind="ExternalInput").ap()
    svec = nc.dram_tensor("svec", [A, 1], f32, kind="ExternalInput").ap()
    sb1v = nc.dram_tensor("sb1v", [A, 1], f32, kind="ExternalInput").ap()
    ttv = nc.dram_tensor("ttv", [A, 1], f32, kind="ExternalInput").ap()
    out = nc.dram_tensor("out", [128, 16], f32, kind="ExternalOutput").ap()

    with tile.TileContext(nc) as tc, ExitStack() as ctx:
        consts = ctx.enter_context(tc.tile_pool(name="consts", bufs=1))
        xpool = ctx.enter_context(tc.tile_pool(name="xpool", bufs=2))
        fpool = ctx.enter_context(tc.tile_pool(name="fpool", bufs=2))
        epool = ctx.enter_context(tc.tile_pool(name="epool", bufs=8))
        vpool = ctx.enter_context(tc.tile_pool(name="vpool", bufs=4))
        ppool = ctx.enter_context(tc.tile_pool(name="ppool", bufs=4))
        accp = ctx.enter_context(tc.tile_pool(name="accp", bufs=2))
        smalls = ctx.enter_context(tc.tile_pool(name="smalls", bufs=8))
        outp = ctx.enter_context(tc.tile_pool(name="outp", bufs=2))
        ph = ctx.enter_context(tc.tile_pool(name="ph", bufs=2, space="PSUM"))
        pa = ctx.enter_context(tc.tile_pool(name="pa", bufs=2, space="PSUM"))
        pz = ctx.enter_context(tc.tile_pool(name="pz", bufs=1, space="PSUM"))

        def load_const(ap_in, shape, dt, name):
            t_ = consts.tile(shape, dt, name=name, tag=name)
            nc.sync.dma_start(t_[:], ap_in)
            return t_

        w1aT_sb = load_const(w1aT, [128, CC * A], xdt, "w1aT_sb")
        w2T_sb = load_const(w2T, [A, C], xdt, "w2T_sb")
        w1bT_sb = load_const(w1bT, [128, CC * A], f32, "w1bT_sb")
        w1cT_sb = load_const(w1cT, [128, CC * A], f32, "w1cT_sb")
        svec_sb = load_const(svec, [A, 1], f32, "svec_sb")
        sb1_sb = load_const(sb1v, [A, 1], f32, "sb1_sb")
        ttv_sb = load_const(ttv, [A, 1], f32, "ttv_sb")
        misc_sb = load_const(misc, [128, 4], f32, "misc_sb")
        invT8_sb = load_const(invT8, [128, 8], f32, "invT8_sb")
        cnt8_sb = load_const(cnt8, [128, 8], f32, "cnt8_sb")

        def newton_sqrt(pool, var_t, w, iters, out=None):
            """Elementwise sqrt of a [128, w] fp32 tile (values >= EPS)."""
            yb = pool.tile([128, w], i32, tag="nt_yb")
            nc.vector.tensor_scalar(
                out=yb[:], in0=var_t[:].bitcast(i32), scalar1=-0.5,
                scalar2=RSQRT_MAGIC, op0=Alu.mult, op1=Alu.add)
            y = yb[:].bitcast(f32)
            for _ in range(iters):
                t1 = pool.tile([128, w], f32, tag="nt_t1")
                nc.vector.tensor_tensor(out=t1[:], in0=y, in1=y, op=Alu.mult)
                nc.vector.tensor_tensor(out=t1[:], in0=t1[:], in1=var_t[:],
                                        op=Alu.mult)
                nc.vector.tensor_scalar(
                    out=t1[:], in0=t1[:], scalar1=-0.5, scalar2=1.5,
                    op0=Alu.mult, op1=Alu.add)
                yn = pool.tile([128, w], f32, tag="nt_yn")
                nc.vector.tensor_tensor(out=yn[:], in0=y, in1=t1[:],
                                        op=Alu.mult)
                y = yn[:]
            if out is None:
                r = pool.tile([128, w], f32, tag="nt_r")
                out = r[:]
            nc.vector.tensor_tensor(out=out, in0=var_t[:], in1=y, op=Alu.mult)
            return out

        xconst = None
        if OPTS["probe"] == 2:
            xconst = []
            for sl in range(2):
                for cc in range(CC):
                    xt = consts.tile([128, widths[sl]], xdt,
                                     name=f"xc{sl}_{cc}", tag=f"xc{sl}_{cc}")
                    nc.sync.dma_start(xt[:],
                                      xs[sl][cc * 128:(cc + 1) * 128, :])
                    xconst.append(xt)

        def stage_prep(pipe, iv):
            """Loads + sum-x + stats for one logical iteration. Runs one
            pipeline tick ahead of stage_blocks so bias/e08 are ready
            before the block phase starts."""
            if OPTS["probe"] == 2:
                xf_all, xret = list(xconst), []
            elif OPTS["xmerge"]:
                xf_all, xret = [], []
                for sl in range(2):
                    xt3 = pipe.intermediate_tile([128, CC, widths[sl]], xdt,
                                                 name=f"x3_{sl}", bufs=2)
                    xin = xs[sl].rearrange("(cc p) w -> p cc w", p=128)
                    nc.sync.dma_start(xt3[:], xin)
                    xret.append(xt3)
                    xf_all.extend(xt3[:, cc] for cc in range(CC))
            else:
                xf_all = []
                for sl in range(2):
                    xbufs = 4 if (sl == 1 and OPTS["x1_bufs4"]) else 2
                    for cc in range(CC):
                        xt = pipe.intermediate_tile([128, widths[sl]], xdt,
                                                    name=f"xf{sl}{cc}",
                                                    bufs=xbufs)
                        eng = (nc.scalar
                               if (OPTS["dma_rings"] == 2 and cc >= 2)
                               else nc.sync)
                        eng.dma_start(xt[:],
                                      xs[sl][cc * 128:(cc + 1) * 128, :])
                        xf_all.append(xt)
                xret = xf_all

            for sl in range(2):
                W = widths[sl]
                xf = xf_all[sl * CC:(sl + 1) * CC]
                if OPTS["sumx_mode"] == "mix" and sl == 1:
                    # slot1 sum-x rides the idle Act engine: Copy+accum.
                    xs3 = accp.tile([128, CC, 1], f32, tag=f"xs3_{sl}",
                                    name=f"xs3_{sl}")
                    for cc in range(CC):
                        t0 = fpool.tile([128, W], xdt, tag=f"acpy_{sl}",
                                        name=f"acpy_{sl}_{cc}")
                        nc.scalar.activation(
                            out=t0[:], in_=xf[cc][:, :W], func=Act.Copy,
                            accum_out=xs3[:, cc, 0:1])
                elif OPTS["sumx_mode"] in ("pool", "mix"):
                    # GpSimd halving folds W -> W/4, then one short DVE
                    # cache-reduce: moves ~2/3 of the sum-x stream off DVE.
                    h1, h2 = W // 2, W // 4
                    xs3 = accp.tile([128, CC, 1], f32, tag=f"xs3_{sl}",
                                    name=f"xs3_{sl}")
                    for cc in range(CC):
                        t1 = fpool.tile([128, h1], xdt, tag=f"sxf1_{sl}",
                                        name=f"sxf1_{sl}_{cc}")
                        nc.gpsimd.tensor_tensor(
                            out=t1[:], in0=xf[cc][:, :h1],
                            in1=xf[cc][:, h1:W], op=Alu.add)
                        t2 = fpool.tile([128, h2], xdt, tag=f"sxf2_{sl}",
                                        name=f"sxf2_{sl}_{cc}")
                        nc.gpsimd.tensor_tensor(
                            out=t2[:], in0=t1[:, :h2], in1=t1[:, h2:h1],
                            op=Alu.add)
                        scr = ppool.tile([128, 2 * BLK], xdt, tag="sxout",
                                         name=f"sx{sl}_{cc}")
                        nc.vector.tensor_scalar(
                            out=scr[:, :h2], in0=t2[:], scalar1=0.0,
                            scalar2=None, op0=Alu.bypass, op1=Alu.add,
                            accum_out=xs3[:, cc, 0:1])
                else:
                    xs3 = accp.tile([128, CC, npair[sl]], f32,
                                    tag=f"xs3_{sl}", name=f"xs3_{sl}")
                    for cc in range(CC):
                        for p in range(npair[sl]):
                            w = min(2 * BLK, W - p * 2 * BLK)
                            scr = ppool.tile([128, 2 * BLK], xdt, tag="sxout",
                                             name=f"sx{sl}_{cc}_{p}")
                            nc.vector.tensor_scalar(
                                out=scr[:, :w],
                                in0=xf[cc][:, p * 2 * BLK:p * 2 * BLK + w],
                                scalar1=0.0, scalar2=None, op0=Alu.bypass,
                                op1=Alu.add, accum_out=xs3[:, cc, p:p + 1])
                if sl == 0:
                    xs3_0 = xs3
                else:
                    xs3_1 = xs3

            # fused two-slot stats: mg8/sg8 -> per-slot cvec/bias/e0
            mg8 = smalls.tile([128, 8], f32, tag="mg8", name="mg8")
            nc.vector.tensor_reduce(out=mg8[:, 0:4], in_=xs3_0[:],
                                    axis=mybir.AxisListType.X, op=Alu.add)
            nc.vector.tensor_reduce(out=mg8[:, 4:8], in_=xs3_1[:],
                                    axis=mybir.AxisListType.X, op=Alu.add)
            nc.vector.tensor_tensor(out=mg8[:], in0=mg8[:], in1=invT8_sb[:],
                                    op=Alu.mult)
            # vg8 = max(mg8 - mg8^2, EPS) in two fused ops:
            # t = (mg8 - 1) * mg8 = -(mg8 - mg8^2); vg8 = max(-t, EPS)
            vg8 = smalls.tile([128, 8], f32, tag="vg8", name="vg8")
            nc.vector.scalar_tensor_tensor(out=vg8[:], in0=mg8[:],
                                           scalar=1.0, in1=mg8[:],
                                           op0=Alu.subtract, op1=Alu.mult)
            nc.vector.tensor_scalar(out=vg8[:], in0=vg8[:], scalar1=-1.0,
                                    scalar2=EPS, op0=Alu.mult, op1=Alu.max)
            sg8 = newton_sqrt(smalls, vg8, 8, OPTS["stats_newton"])
            e08 = pipe.intermediate_tile([128, 8], f32, name="e08")
            biases = []
            for sl in range(2):
                c_ps = pz.tile([A, 1], f32, tag="c_ps", name=f"c_ps{sl}")
                for cc in range(CC):
                    nc.tensor.matmul(
                        c_ps[:], w1bT_sb[:, cc * A:(cc + 1) * A],
                        mg8[:, sl * 4 + cc:sl * 4 + cc + 1],
                        start=(cc == 0), stop=False)
                for cc in range(CC):
                    nc.tensor.matmul(
                        c_ps[:], w1cT_sb[:, cc * A:(cc + 1) * A],
                        sg8[:, sl * 4 + cc:sl * 4 + cc + 1],
                        start=False, stop=(cc == CC - 1))
                bias = pipe.intermediate_tile([A, 1], f32, name=f"bias{sl}")
                nc.scalar.activation(out=bias[:], in_=c_ps[:], func=Act.Identity,
                                     bias=sb1_sb[:, 0:1])
                # hfin0 = max(tanh(bias), tanh(t)): the h==0 column value
                v0 = smalls.tile([A, 1], xdt, tag="v0", name=f"v0{sl}")
                nc.scalar.activation(out=v0[:], in_=bias[:], func=Act.Tanh)
                h0 = smalls.tile([A, 1], xdt, tag="h0", name=f"h0{sl}")
                nc.vector.tensor_scalar(out=h0[:], in0=v0[:],
                                        scalar1=ttv_sb[:, 0:1], scalar2=None,
                                        op0=Alu.max)
                z_ps = pz.tile([128, CC], f32, tag="z_ps", name=f"z_ps{sl}")
                for cc in range(CC):
                    nc.tensor.matmul(
                        z_ps[:, cc:cc + 1], w2T_sb[:, cc * 128:(cc + 1) * 128],
                        h0[:], start=True, stop=True)
                nc.scalar.activation(out=e08[:, sl * 4:sl * 4 + 4],
                                     in_=z_ps[:], func=Act.Exp)
                biases.append(bias)
            return tuple(xret) + (biases[0], biases[1], e08)

        def stage_blocks(pipe, iv, state):
            if OPTS["probe"] == 2:
                xf_by_slot = [xconst[sl * CC:(sl + 1) * CC]
                              for sl in range(2)]
                nx = 0
            elif OPTS["xmerge"]:
                xf_by_slot = [[state[sl][:, cc] for cc in range(CC)]
                              for sl in range(2)]
                nx = 2
            else:
                xf_by_slot = [list(state[sl * CC:(sl + 1) * CC])
                              for sl in range(2)]
                nx = 2 * CC
            biases = [state[nx], state[nx + 1]]
            e08 = state[nx + 2]
            st = [{} for _ in range(2)]
            for sl in range(2):
                st[sl]["xf"] = xf_by_slot[sl]
                st[sl]["bias"] = biases[sl]
                st[sl]["se3"] = pipe.intermediate_tile(
                    [128, CC, npair[sl]], f32, name=f"se3_{sl}")
                st[sl]["sp3"] = pipe.intermediate_tile(
                    [128, CC, npair[sl]], f32, name=f"sp3_{sl}")
            if OPTS["probe"]:   # timing probe: skip the block phase
                for sl in range(2):
                    nc.vector.memset(st[sl]["se3"][:], 1.0)
                    nc.vector.memset(st[sl]["sp3"][:], 1.0)
                tmp8 = pipe.intermediate_tile([128, 8], f32, name="tmp8")
                nc.vector.tensor_tensor(out=tmp8[:], in0=e08[:],
                                        in1=cnt8_sb[:], op=Alu.mult)
                return (st[0]["se3"], st[0]["sp3"], st[1]["se3"],
                        st[1]["sp3"], tmp8)

            # ---- block pipeline, slots interleaved
            hfin_by_pair = {}

            def emit_head(sl, p):
                """h matmuls + tanh + hfin for one pair (producer side)."""
                W, xf, bias = widths[sl], st[sl]["xf"], st[sl]["bias"]
                b0 = 2 * p
                bs = [b for b in (b0, b0 + 1) if b < nblk[sl]]
                wseg = min(2 * BLK, W - b0 * BLK)
                hfin = vpool.tile([A, 2 * BLK], xdt, tag="hfin",
                                  name=f"hf{sl}_{p}")
                v = vpool.tile([A, 2 * BLK], xdt, tag="v", name=f"v{sl}_{p}")
                for b in bs:
                    off = (b - b0) * BLK
                    w = min(BLK, W - b * BLK)
                    h_ps = ph.tile([A, BLK], f32, tag="h_ps",
                                   name=f"h{sl}_{b}")
                    for cc in range(CC):
                        nc.tensor.matmul(
                            h_ps[:, :w], w1aT_sb[:, cc * A:(cc + 1) * A],
                            xf[cc][:, b * BLK:b * BLK + w],
                            start=(cc == 0), stop=(cc == CC - 1))
                    nc.scalar.activation(
                        out=v[:, off:off + w], in_=h_ps[:, :w], func=Act.Tanh,
                        bias=bias[:, 0:1], scale=svec_sb[:, 0:1])
                nc.vector.tensor_scalar(
                    out=hfin[:, :wseg], in0=v[:, :wseg],
                    scalar1=ttv_sb[:, 0:1], scalar2=None, op0=Alu.max)
                hfin_by_pair[(sl, p)] = hfin

            def emit_tail(sl, p):
                """a matmuls + exp/sum-e + fused e*x reduce (consumer side)."""
                W, xf = widths[sl], st[sl]["xf"]
                se3, sp3 = st[sl]["se3"], st[sl]["sp3"]
                b0 = 2 * p
                bs = [b for b in (b0, b0 + 1) if b < nblk[sl]]
                wseg = min(2 * BLK, W - b0 * BLK)
                hfin = hfin_by_pair.pop((sl, p))
                e_pair = [epool.tile([128, 2 * BLK], xdt, tag="e",
                                     name=f"e{sl}_{p}_{i}") for i in range(CC)]
                for cc in range(CC):
                    a_ps = pa.tile([128, 2 * BLK], f32, tag="a_ps",
                                   name=f"a{sl}_{p}_{cc}")
                    for b in bs:
                        off = (b - b0) * BLK
                        w = min(BLK, W - b * BLK)
                        nc.tensor.matmul(
                            a_ps[:, off:off + w],
                            w2T_sb[:, cc * 128:(cc + 1) * 128],
                            hfin[:, off:off + w], start=True, stop=True)
                    route = exp_route[(sl, p, cc)]
                    eslice = e_pair[cc][:, :wseg]
                    if route == "act" and (sl, p, cc) not in sedve:
                        nc.scalar.activation(
                            out=eslice, in_=a_ps[:, :wseg], func=Act.Exp,
                            accum_out=se3[:, cc, p:p + 1])
                    elif route == "act":
                        nc.scalar.activation(
                            out=eslice, in_=a_ps[:, :wseg], func=Act.Exp)
                        scr2 = ppool.tile([128, 2 * BLK], xdt, tag="seout",
                                          name=f"sa{sl}_{p}_{cc}")
                        nc.vector.tensor_scalar(
                            out=scr2[:, :wseg], in0=eslice, scalar1=0.0,
                            scalar2=None, op0=Alu.bypass, op1=Alu.add,
                            accum_out=se3[:, cc, p:p + 1])
                    else:
                        nc.vector.tensor_scalar(
                            out=eslice.bitcast(i16), in0=a_ps[:, :wseg],
                            scalar1=SCH_A, scalar2=SCH_B,
                            op0=Alu.mult, op1=Alu.add)
                        scr2 = ppool.tile([128, 2 * BLK], xdt, tag="seout",
                                          name=f"se{sl}_{p}_{cc}")
                        nc.vector.tensor_scalar(
                            out=scr2[:, :wseg], in0=eslice, scalar1=0.0,
                            scalar2=None, op0=Alu.bypass, op1=Alu.add,
                            accum_out=se3[:, cc, p:p + 1])
                for cc in range(CC):
                    scr = ppool.tile([128, 2 * BLK], xdt, tag="pout",
                                     name=f"p{sl}_{p}_{cc}")
                    if OPTS["use_stt"]:
                        # fused product + free-dim reduce in ONE 1x DVE op
                        nc.vector.scalar_tensor_tensor(
                            out=scr[:, :wseg], in0=e_pair[cc][:, :wseg],
                            scalar=1.0,
                            in1=xf[cc][:, b0 * BLK:b0 * BLK + wseg],
                            op0=Alu.mult, op1=Alu.mult,
                            accum_out=sp3[:, cc, p:p + 1])
                    else:
                        nc.vector.tensor_tensor(
                            out=scr[:, :wseg], in0=e_pair[cc][:, :wseg],
                            in1=xf[cc][:, b0 * BLK:b0 * BLK + wseg],
                            op=Alu.mult)
                        scr2 = ppool.tile([128, 2 * BLK], xdt, tag="seout",
                                          name=f"ps{sl}_{p}_{cc}")
                        nc.vector.tensor_scalar(
                            out=scr2[:, :wseg], in0=scr[:, :wseg],
                            scalar1=0.0, scalar2=None, op0=Alu.bypass,
                            op1=Alu.add, accum_out=sp3[:, cc, p:p + 1])

            # software-pipelined emission: pair k+1's h/tanh/hfin enters the
            # engine queues BEFORE pair k's a/exp/e*x, so PE and Act never
            # head-of-line block on the DVE->PE hfin dependency.
            LEAD = 2
            for kk in range(min(LEAD, len(emit_order))):
                emit_head(*emit_order[kk])
            for k in range(len(emit_order)):
                if k + LEAD < len(emit_order):
                    emit_head(*emit_order[k + LEAD])
                emit_tail(*emit_order[k])
            # tail correction term cnt * e0, owned by this stage so it can
            # flow to stage_final
            tmp8 = pipe.intermediate_tile([128, 8], f32, name="tmp8")
            nc.vector.tensor_tensor(out=tmp8[:], in0=e08[:], in1=cnt8_sb[:],
                                    op=Alu.mult)
            return (st[0]["se3"], st[0]["sp3"], st[1]["se3"], st[1]["sp3"],
                    tmp8)

        def stage_final(pipe, iv, state):
            se3 = [state[0], state[2]]
            sp3 = [state[1], state[3]]
            tmp8 = state[4]
            out_sb = outp.tile([128, 16], f32, tag="out_sb")
            se8 = smalls.tile([128, 8], f32, tag="se8", name="se8")
            sp8 = smalls.tile([128, 8], f32, tag="sp8", name="sp8")
            if True:
                for sl in range(2):
                    nc.vector.tensor_reduce(
                        out=se8[:, sl * 4:sl * 4 + 4], in_=se3[sl][:],
                        axis=mybir.AxisListType.X, op=Alu.add)
                    nc.vector.tensor_reduce(
                        out=sp8[:, sl * 4:sl * 4 + 4], in_=sp3[sl][:],
                        axis=mybir.AxisListType.X, op=Alu.add)
                # subtract the tail contribution cnt * e0 from sum(e)
                nc.vector.tensor_tensor(out=se8[:], in0=se8[:], in1=tmp8[:],
                                        op=Alu.subtract)
                rec = smalls.tile([128, 8], f32, tag="rec8", name="rec8")
                nc.vector.reciprocal(out=rec[:], in_=se8[:])
                mean_o = out_sb[:, 0:8]
                nc.vector.tensor_tensor(out=mean_o, in0=sp8[:], in1=rec[:],
                                        op=Alu.mult)
                var_t = smalls.tile([128, 8], f32, tag="var8", name="var8")
                nc.vector.scalar_tensor_tensor(out=var_t[:], in0=mean_o,
                                               scalar=1.0, in1=mean_o,
                                               op0=Alu.subtract, op1=Alu.mult)
                nc.vector.tensor_scalar(out=var_t[:], in0=var_t[:],
                                        scalar1=-1.0, scalar2=EPS,
                                        op0=Alu.mult, op1=Alu.max)
                newton_sqrt(smalls, var_t, 8, OPTS["final_newton"],
                            out=out_sb[:, 8:16])
            nc.sync.dma_start(out, out_sb[:])

        if loop:
            reps_sb = consts.tile([1, 1], i32, name="reps_sb", tag="reps_sb")
            nc.sync.dma_start(reps_sb[:], reps)
            regs = nc.alloc_registers("reps_regs")
            nc.regs_load(regs, reps_sb[:1, :1])
            rv = nc.snap(regs, donate=True)
            hints = (tuple(mybir.ALL_ENGINES) if OPTS["loop_hints"] else ())
            tc.For_i_pipelined([stage_prep, stage_blocks, stage_final],
                               0, rv, 1, unroll=OPTS["pipe_unroll"],
                               hint_engines=hints,
                               staggered_reset=bool(OPTS["stagger"]))
        else:
            class _FakePipe:
                """Pipelined-emission stand-in for PipelineAllocator."""
                def intermediate_tile(self, shape, dtype, name=None,
                                      bufs=None, **kw):
                    return xpool.tile(shape, dtype, tag=f"pp_{name}",
                                      name=name, bufs=(bufs or 4), **kw)
            fp = _FakePipe()
            preps, blks = {}, {}
            for t in range(unroll + 2):
                if t >= 2:
                    stage_final(fp, 0, blks.pop(t - 2))
                if 1 <= t <= unroll:
                    blks[t - 1] = stage_blocks(fp, 0, preps.pop(t - 1))
                if t < unroll:
                    preps[t] = stage_prep(fp, 0)

    nc.compile()
    return nc


# ---------------------------------------------------------------- interface

_PROGRAM_CACHE = {}


def _get_program(widths, tmins, loop=False):
    key = (tuple(widths), tuple(tmins), loop, tuple(sorted(
        (k, v) for k, v in OPTS.items())))
    if key not in _PROGRAM_CACHE:
        _PROGRAM_CACHE[key] = _build_program(widths, tmins, loop=loop)
    return _PROGRAM_CACHE[key]


def _prepare(inputs, loop=False):
    in_maps, metas, widths, tmins = _host_prep(**inputs)
    nc = _get_program(widths, tmins, loop=loop)
    return nc, in_maps, metas


def _gather(results, metas):
    pooled = np.zeros((N, 2 * C, 1), dtype=np.float32)
    for core in range(NCORES):
        o = np.asarray(results[core]["out"])   # [128, 16]
        for sl in range(2):
            n, _T = metas[core][sl]
            pooled[n, :C, 0] = o[:, sl * 4:sl * 4 + 4].T.reshape(C)
            pooled[n, C:, 0] = o[:, 8 + sl * 4:8 + sl * 4 + 4].T.reshape(C)
    return pooled


def kernel(**inputs):
    from concourse.bass_utils import run_bass_kernel_spmd
    nc, in_maps, metas = _prepare(inputs)
    res = run_bass_kernel_spmd(nc, in_maps, core_ids=list(range(NCORES)))
    return _gather(res.results, metas)

